# revision 4
# baseline (speedup 1.0000x reference)
"""Trainium2 Bass kernel for CubicShapeFunction (500k particles, 4^3 cubic
B-spline stencil), data-parallel over 8 NeuronCores.

Math: rel = pos*20, f = frac(rel) in [0,1). The stencil distance for offset
o in {0,1,2,3} is f+1-o, which falls in spline branch c4,c3,c2,c1
respectively, so every basis/dbasis value is a fixed cubic in f (no
selects):

  o=0: B=(1-f)^3/6            db/h: -f^2/2 + f - 1/2
  o=1: B=f^3/2 - f^2 + 2/3    db/h:  3f^2/2 - 2f
  o=2: B=-f^3/2+f^2/2+f/2+1/6 db/h: -3f^2/2 + f + 1/2
  o=3: B=f^3/6                db/h:  f^2/2

Per-core layout: 128 particles on partitions x fp=31 particles per
partition per tile; broadcast access patterns evaluate all 12 basis and 12
dbasis values per particle in 10 tensor ops, then outer-product multiplies
build shapef [*,64] and grad [*,64,3] in SBUF in the exact DRAM layout, so
output DMAs are fully contiguous (1 MiB / 3 MiB per tile).

  shapef[n, k]  = B0[oi]*B1[oj]*B2[ok],        k = oi*16+oj*4+ok
  grad[n, k, d] = DBd[od] * prod_{d'!=d} Bd'[od']
"""

import numpy as np

import concourse.bass as bass  # noqa: F401  (bass types used via bacc/tile)
import concourse.bacc as bacc
import concourse.tile as tile
from concourse import mybir
from concourse import bass_utils

F32 = mybir.dt.float32
INV_CELL = 20.0

N_TOTAL = 500_000
N_CORES = 8
N_PER_CORE = N_TOTAL // N_CORES          # 62500
FP = 31                                   # particles per partition per tile
ROWS_PER_TILE = 128 * FP                  # 3968
NPC = 63488                               # padded per-core rows = 16 tiles
NTILES = NPC // ROWS_PER_TILE

_A3 = [-1 / 6, 1 / 2, -1 / 2, 1 / 6]
_A2 = [1 / 2, -1, 1 / 2, 0]
_A1 = [-1 / 2, 0, 1 / 2, 0]
_A0 = [1 / 6, 2 / 3, 1 / 6, 0]
_D2 = [-0.5 * INV_CELL, 1.5 * INV_CELL, -1.5 * INV_CELL, 0.5 * INV_CELL]
_D1 = [1.0 * INV_CELL, -2.0 * INV_CELL, 1.0 * INV_CELL, 0.0]
_D0 = [-0.5 * INV_CELL, 0.0, 0.5 * INV_CELL, 0.0]


def coef_array() -> np.ndarray:
    c = np.array([_A3, _A2, _A1, _A0, _D2, _D1, _D0], dtype=np.float32)  # [7,4]
    return np.broadcast_to(c, (128, 7, 4)).copy()


def build_module(n_rows: int = NPC, fp: int = FP, num_devices: int = N_CORES,
                 enable_asserts: bool = False):
    P = 128
    rows_per_tile = P * fp
    assert n_rows % rows_per_tile == 0
    ntiles = n_rows // rows_per_tile

    nc = bacc.Bacc(
        "TRN2",
        target_bir_lowering=False,
        debug=False,
        enable_asserts=enable_asserts,
        num_devices=num_devices,
    )
    pos_d = nc.dram_tensor("pos", [n_rows, 3], F32, kind="ExternalInput").ap()
    coef_d = nc.dram_tensor("coef", [P, 7, 4], F32, kind="ExternalInput").ap()
    shapef_d = nc.dram_tensor("shapef", [n_rows, 64], F32, kind="ExternalOutput").ap()
    grad_d = nc.dram_tensor("grad", [n_rows, 192], F32, kind="ExternalOutput").ap()

    pos_v = pos_d.rearrange("(t p q) d -> t p (q d)", t=ntiles, p=P)
    sh_v = shapef_d.rearrange("(t p q) w -> t p (q w)", t=ntiles, p=P)
    gr_v = grad_d.rearrange("(t p q) w -> t p (q w)", t=ntiles, p=P)

    with tile.TileContext(nc) as tc:
        with (
            tc.tile_pool(name="const", bufs=1) as constp,
            tc.tile_pool(name="inp", bufs=3) as inp,
            tc.tile_pool(name="bdb", bufs=3) as bdbp,
            tc.tile_pool(name="pp", bufs=3) as ppp,
            tc.tile_pool(name="sg", bufs=2) as sgp,
        ):
            coef_t = constp.tile([P, 7, 4], F32)
            nc.sync.dma_start(out=coef_t, in_=coef_d)

            def cview(i):
                return (
                    coef_t[:, i, :]
                    .unsqueeze(1)
                    .unsqueeze(2)
                    .broadcast_to([P, fp, 3, 4])
                )

            for it in range(ntiles):
                pos_t = inp.tile([P, fp, 3], F32, tag="pos")
                nc.sync.dma_start(out=pos_t.rearrange("p q d -> p (q d)"), in_=pos_v[it])

                # rel = pos*20 in [2,18); f = rel - floor(rel).
                # floor via the fp32 magic-number round trick (+2^23,-2^23 gives
                # round-to-nearest; subtract 1 where round > rel). Exact in f32.
                rel = inp.tile([P, fp, 3], F32, tag="rel")
                fl = inp.tile([P, fp, 3], F32, tag="fl")
                gt = inp.tile([P, fp, 3], F32, tag="gt")
                fr = inp.tile([P, fp, 3], F32, tag="fr")
                MAGIC = float(2 ** 23)
                nc.vector.tensor_scalar(
                    out=rel, in0=pos_t, scalar1=INV_CELL, scalar2=None,
                    op0=mybir.AluOpType.mult,
                )
                nc.vector.tensor_scalar(
                    out=fl, in0=rel, scalar1=MAGIC, scalar2=MAGIC,
                    op0=mybir.AluOpType.add, op1=mybir.AluOpType.subtract,
                )
                nc.vector.tensor_tensor(out=gt, in0=fl, in1=rel, op=mybir.AluOpType.is_gt)
                nc.vector.tensor_tensor(out=fl, in0=fl, in1=gt, op=mybir.AluOpType.subtract)
                nc.vector.tensor_tensor(out=fr, in0=rel, in1=fl, op=mybir.AluOpType.subtract)
                frv = fr.unsqueeze(3).broadcast_to([P, fp, 3, 4])

                B = bdbp.tile([P, fp, 3, 4], F32, tag="B")
                DB = bdbp.tile([P, fp, 3, 4], F32, tag="DB")
                nc.vector.tensor_tensor(out=B, in0=cview(0), in1=frv, op=mybir.AluOpType.mult)
                nc.vector.tensor_tensor(out=B, in0=B, in1=cview(1), op=mybir.AluOpType.add)
                nc.vector.tensor_tensor(out=B, in0=B, in1=frv, op=mybir.AluOpType.mult)
                nc.vector.tensor_tensor(out=B, in0=B, in1=cview(2), op=mybir.AluOpType.add)
                nc.vector.tensor_tensor(out=B, in0=B, in1=frv, op=mybir.AluOpType.mult)
                nc.vector.tensor_tensor(out=B, in0=B, in1=cview(3), op=mybir.AluOpType.add)
                nc.gpsimd.tensor_tensor(out=DB, in0=cview(4), in1=frv, op=mybir.AluOpType.mult)
                nc.gpsimd.tensor_tensor(out=DB, in0=DB, in1=cview(5), op=mybir.AluOpType.add)
                nc.gpsimd.tensor_tensor(out=DB, in0=DB, in1=frv, op=mybir.AluOpType.mult)
                nc.gpsimd.tensor_tensor(out=DB, in0=DB, in1=cview(6), op=mybir.AluOpType.add)

                def bx(d):
                    return B[:, :, d, :]

                def dbx(d):
                    return DB[:, :, d, :]

                # pair products: P01=B0xB1, P12=B1xB2, Q12=DB1xB2 (all [i,j] / [j,k])
                P01 = ppp.tile([P, fp, 4, 4], F32, tag="P01")
                P12 = ppp.tile([P, fp, 4, 4], F32, tag="P12")
                Q12 = ppp.tile([P, fp, 4, 4], F32, tag="Q12")
                nc.gpsimd.tensor_tensor(
                    out=P01,
                    in0=bx(0).unsqueeze(3).broadcast_to([P, fp, 4, 4]),
                    in1=bx(1).unsqueeze(2).broadcast_to([P, fp, 4, 4]),
                    op=mybir.AluOpType.mult,
                )
                nc.gpsimd.tensor_tensor(
                    out=P12,
                    in0=bx(1).unsqueeze(3).broadcast_to([P, fp, 4, 4]),
                    in1=bx(2).unsqueeze(2).broadcast_to([P, fp, 4, 4]),
                    op=mybir.AluOpType.mult,
                )
                nc.gpsimd.tensor_tensor(
                    out=Q12,
                    in0=dbx(1).unsqueeze(3).broadcast_to([P, fp, 4, 4]),
                    in1=bx(2).unsqueeze(2).broadcast_to([P, fp, 4, 4]),
                    op=mybir.AluOpType.mult,
                )

                S = sgp.tile([P, fp, 16, 4], F32, tag="S")
                nc.vector.tensor_tensor(
                    out=S,
                    in0=P01.rearrange("p q i j -> p q (i j)").unsqueeze(3).broadcast_to([P, fp, 16, 4]),
                    in1=bx(2).unsqueeze(2).broadcast_to([P, fp, 16, 4]),
                    op=mybir.AluOpType.mult,
                )

                G = sgp.tile([P, fp, 64, 3], F32, tag="G")
                g0 = G.rearrange("p q (i m) d -> p q i m d", i=4)[:, :, :, :, 0]
                nc.vector.tensor_tensor(
                    out=g0,
                    in0=dbx(0).unsqueeze(3).broadcast_to([P, fp, 4, 16]),
                    in1=P12.rearrange("p q j k -> p q (j k)").unsqueeze(2).broadcast_to([P, fp, 4, 16]),
                    op=mybir.AluOpType.mult,
                )
                g1 = G.rearrange("p q (i m) d -> p q i m d", i=4)[:, :, :, :, 1]
                nc.vector.tensor_tensor(
                    out=g1,
                    in0=bx(0).unsqueeze(3).broadcast_to([P, fp, 4, 16]),
                    in1=Q12.rearrange("p q j k -> p q (j k)").unsqueeze(2).broadcast_to([P, fp, 4, 16]),
                    op=mybir.AluOpType.mult,
                )
                g2 = G.rearrange("p q (m k) d -> p q m k d", m=16)[:, :, :, :, 2]
                nc.vector.tensor_tensor(
                    out=g2,
                    in0=dbx(2).unsqueeze(2).broadcast_to([P, fp, 16, 4]),
                    in1=P01.rearrange("p q i j -> p q (i j)").unsqueeze(3).broadcast_to([P, fp, 16, 4]),
                    op=mybir.AluOpType.mult,
                )

                nc.sync.dma_start(out=sh_v[it], in_=S.rearrange("p q m k -> p (q m k)"))
                nc.sync.dma_start(out=gr_v[it], in_=G.rearrange("p q w d -> p (q w d)"))

    nc.compile()
    return nc


_MODULE_CACHE = {}


def _get_module():
    if "nc" not in _MODULE_CACHE:
        _MODULE_CACHE["nc"] = build_module()
    return _MODULE_CACHE["nc"]


def run(position_stack: np.ndarray, trace: bool = False):
    """Run on 8 NeuronCores. Returns ((shapef, grad), BassKernelResults)."""
    pos = np.ascontiguousarray(np.asarray(position_stack, dtype=np.float32))
    assert pos.shape == (N_TOTAL, 3), pos.shape

    nc = _get_module()
    coef = coef_array()
    in_maps = []
    for c in range(N_CORES):
        sl = pos[c * N_PER_CORE : (c + 1) * N_PER_CORE]
        padded = np.full((NPC, 3), 0.5, dtype=np.float32)
        padded[:N_PER_CORE] = sl
        in_maps.append({"pos": padded, "coef": coef})

    res = bass_utils.run_bass_kernel_spmd(
        nc, in_maps, core_ids=list(range(N_CORES)), trace=trace,
    )

    shapef = np.empty((N_TOTAL, 64), dtype=np.float32)
    grad = np.empty((N_TOTAL, 64, 3), dtype=np.float32)
    for c, r in enumerate(res.results):
        shapef[c * N_PER_CORE : (c + 1) * N_PER_CORE] = r["shapef"][:N_PER_CORE]
        grad[c * N_PER_CORE : (c + 1) * N_PER_CORE] = r["grad"][:N_PER_CORE].reshape(
            N_PER_CORE, 64, 3
        )
    return (shapef, grad), res


def kernel(position_stack: np.ndarray):
    (shapef, grad), _ = run(position_stack, trace=False)
    return shapef, grad


# revision 7
# speedup vs baseline: 1.1994x; 1.1994x over previous
"""Trainium2 Bass kernel for CubicShapeFunction (500k particles, 4^3 cubic
B-spline stencil), data-parallel over 8 NeuronCores.

Math: rel = pos*20, f = frac(rel) in [0,1), g = 1-f. The stencil distance
for offset o in {0,1,2,3} is f+1-o, which falls in spline branch
c4,c3,c2,c1 respectively, so (h = 20):

  B0 = g^3/6          DB0 = -h g^2/2
  B1 = 2/3 - f^2(1-f/2)   DB1 = f * (1.5h f - 2h)
  B2 = 2/3 - g^2(1-g/2)   DB2 = g * (2h - 1.5h g)
  B3 = f^3/6          DB3 = h f^2/2

outputs:
  shapef[n, k]  = B0[oi]*B1[oj]*B2[ok],        k = oi*16+oj*4+ok  (o-index per dim)
  grad[n, k, d] = DBd[od] * prod_{d'!=d} Bd'[od']

Engine split per tile (128 partitions x fp particles each):
  Scalar(ACT): rel, g, squares, all affine basis-column evaluations
  Vector(DVE): floor trick, cubes, two pair-product-free products,
               S(half), G0, G2
  GpSimd:      pair products P01/P12/Q12, G1, S(other half)
All output tiles are laid out so the S and G DMAs are fully contiguous.
"""

import numpy as np

import concourse.bass as bass  # noqa: F401
import concourse.bacc as bacc
import concourse.tile as tile
from concourse import mybir
from concourse import bass_utils

F32 = mybir.dt.float32
H = 20.0
MAGIC = float(2 ** 23)

N_TOTAL = 500_000
N_CORES = 8
N_PER_CORE = N_TOTAL // N_CORES          # 62500
FP = 56                                   # particles per partition per tile
ROWS_PER_TILE = 128 * FP                  # 7168
NTILES = 9
NPC = ROWS_PER_TILE * NTILES              # 64512 padded rows per core


def build_module(n_rows: int = NPC, fp: int = FP, num_devices: int = N_CORES,
                 enable_asserts: bool = False):
    P = 128
    rows_per_tile = P * fp
    assert n_rows % rows_per_tile == 0
    ntiles = n_rows // rows_per_tile

    nc = bacc.Bacc(
        "TRN2",
        target_bir_lowering=False,
        debug=False,
        enable_asserts=enable_asserts,
        num_devices=num_devices,
    )
    # pre-register activation-bias constants (only 0.0/1.0 exist by default)
    for value in (2.0 / 3.0, -2.0 * H, 2.0 * H):
        t = nc.alloc_sbuf_tensor(f"const-f32-{value}", [128, 1], F32)
        nc.gpsimd.memset(t.ap(), value)
        nc.const_aps.aps[(F32, value)] = t.ap()
    nc.all_engine_barrier()

    pos_d = nc.dram_tensor("pos", [n_rows, 3], F32, kind="ExternalInput").ap()
    shapef_d = nc.dram_tensor("shapef", [n_rows, 64], F32, kind="ExternalOutput").ap()
    grad_d = nc.dram_tensor("grad", [n_rows, 192], F32, kind="ExternalOutput").ap()

    pos_v = pos_d.rearrange("(t p q) d -> p t (q d)", t=ntiles, p=P)
    sh_v = shapef_d.rearrange("(t p q) w -> t p (q w)", t=ntiles, p=P)
    gr_v = grad_d.rearrange("(t p q) w -> t p (q w)", t=ntiles, p=P)

    A = mybir.AluOpType
    ID = mybir.ActivationFunctionType.Identity
    SQ = mybir.ActivationFunctionType.Square

    with tile.TileContext(nc) as tc:
        with (
            tc.tile_pool(name="const", bufs=1) as constp,
            tc.tile_pool(name="inp", bufs=2) as inp,
            tc.tile_pool(name="bdb", bufs=2) as bdbp,
            tc.tile_pool(name="pp", bufs=2) as ppp,
            tc.tile_pool(name="sg", bufs=2) as sgp,
        ):
            # all positions for this core in one DMA (6.7KB/partition)
            POS = constp.tile([P, ntiles, fp * 3], F32)
            nc.sync.dma_start(out=POS, in_=pos_v)

            for it in range(ntiles):
                posv = POS[:, it, :].rearrange("p (q d) -> p q d", d=3)

                def t3(tag):
                    return inp.tile([P, fp, 3], F32, tag=tag, name=tag)

                rel, fl, gt, fr, g = t3("rel"), t3("fl"), t3("gt"), t3("fr"), t3("g")
                f2, g2, f3, g3 = t3("f2"), t3("g2"), t3("f3"), t3("g3")
                t1, t2, vv, ww = t3("t1"), t3("t2"), t3("vv"), t3("ww")

                # rel = 20*pos; floor via fp32 magic round (+2^23-2^23, fix up
                # round>rel), f = rel - floor, g = 1 - f. Exact in f32.
                nc.scalar.mul(out=rel, in_=posv, mul=H)
                nc.vector.tensor_scalar(out=fl, in0=rel, scalar1=MAGIC, scalar2=MAGIC,
                                        op0=A.add, op1=A.subtract)
                nc.vector.tensor_tensor(out=gt, in0=fl, in1=rel, op=A.is_gt)
                nc.vector.tensor_tensor(out=fl, in0=fl, in1=gt, op=A.subtract)
                nc.vector.tensor_tensor(out=fr, in0=rel, in1=fl, op=A.subtract)
                nc.scalar.activation(out=g, in_=fr, func=ID, bias=1.0, scale=-1.0)

                # powers
                nc.scalar.activation(out=f2, in_=fr, func=SQ)
                nc.scalar.activation(out=g2, in_=g, func=SQ)
                nc.vector.tensor_tensor(out=f3, in0=f2, in1=fr, op=A.mult)
                nc.vector.tensor_tensor(out=g3, in0=g2, in1=g, op=A.mult)

                B = bdbp.tile([P, fp, 3, 4], F32, tag="B")
                DB = bdbp.tile([P, fp, 3, 4], F32, tag="DB")

                # basis columns
                nc.scalar.mul(out=B[:, :, :, 0], in_=g3, mul=1.0 / 6.0)
                nc.scalar.mul(out=B[:, :, :, 3], in_=f3, mul=1.0 / 6.0)
                nc.scalar.activation(out=t1, in_=fr, func=ID, bias=1.0, scale=-0.5)
                nc.vector.tensor_tensor(out=t1, in0=f2, in1=t1, op=A.mult)
                nc.scalar.activation(out=B[:, :, :, 1], in_=t1, func=ID,
                                     bias=2.0 / 3.0, scale=-1.0)
                nc.scalar.activation(out=t2, in_=g, func=ID, bias=1.0, scale=-0.5)
                nc.vector.tensor_tensor(out=t2, in0=g2, in1=t2, op=A.mult)
                nc.scalar.activation(out=B[:, :, :, 2], in_=t2, func=ID,
                                     bias=2.0 / 3.0, scale=-1.0)

                # dbasis columns
                nc.scalar.mul(out=DB[:, :, :, 0], in_=g2, mul=-0.5 * H)
                nc.scalar.mul(out=DB[:, :, :, 3], in_=f2, mul=0.5 * H)
                nc.scalar.activation(out=vv, in_=fr, func=ID, bias=-2.0 * H, scale=1.5 * H)
                nc.vector.tensor_tensor(out=DB[:, :, :, 1], in0=vv, in1=fr, op=A.mult)
                nc.scalar.activation(out=ww, in_=g, func=ID, bias=2.0 * H, scale=-1.5 * H)
                nc.vector.tensor_tensor(out=DB[:, :, :, 2], in0=ww, in1=g, op=A.mult)

                def bx(d):
                    return B[:, :, d, :]

                def dbx(d):
                    return DB[:, :, d, :]

                # pair products on gpsimd
                P01 = ppp.tile([P, fp, 4, 4], F32, tag="P01")
                P12 = ppp.tile([P, fp, 4, 4], F32, tag="P12")
                Q12 = ppp.tile([P, fp, 4, 4], F32, tag="Q12")
                nc.gpsimd.tensor_tensor(
                    out=P01,
                    in0=bx(0).unsqueeze(3).broadcast_to([P, fp, 4, 4]),
                    in1=bx(1).unsqueeze(2).broadcast_to([P, fp, 4, 4]),
                    op=A.mult)
                nc.gpsimd.tensor_tensor(
                    out=P12,
                    in0=bx(1).unsqueeze(3).broadcast_to([P, fp, 4, 4]),
                    in1=bx(2).unsqueeze(2).broadcast_to([P, fp, 4, 4]),
                    op=A.mult)
                nc.gpsimd.tensor_tensor(
                    out=Q12,
                    in0=dbx(1).unsqueeze(3).broadcast_to([P, fp, 4, 4]),
                    in1=bx(2).unsqueeze(2).broadcast_to([P, fp, 4, 4]),
                    op=A.mult)

                # final products
                S = sgp.tile([P, fp, 16, 4], F32, tag="S")
                p01f = P01.rearrange("p q i j -> p q (i j)")
                # S split: first half (oi=0,1) on vector, second on gpsimd
                nc.vector.tensor_tensor(
                    out=S[:, :, 0:8, :],
                    in0=p01f[:, :, 0:8].unsqueeze(3).broadcast_to([P, fp, 8, 4]),
                    in1=bx(2).unsqueeze(2).broadcast_to([P, fp, 8, 4]),
                    op=A.mult)
                nc.gpsimd.tensor_tensor(
                    out=S[:, :, 8:16, :],
                    in0=p01f[:, :, 8:16].unsqueeze(3).broadcast_to([P, fp, 8, 4]),
                    in1=bx(2).unsqueeze(2).broadcast_to([P, fp, 8, 4]),
                    op=A.mult)

                G = sgp.tile([P, fp, 64, 3], F32, tag="G")
                g0 = G.rearrange("p q (i m) d -> p q i m d", i=4)[:, :, :, :, 0]
                nc.vector.tensor_tensor(
                    out=g0,
                    in0=dbx(0).unsqueeze(3).broadcast_to([P, fp, 4, 16]),
                    in1=P12.rearrange("p q j k -> p q (j k)").unsqueeze(2).broadcast_to([P, fp, 4, 16]),
                    op=A.mult)
                g1 = G.rearrange("p q (i m) d -> p q i m d", i=4)[:, :, :, :, 1]
                nc.gpsimd.tensor_tensor(
                    out=g1,
                    in0=bx(0).unsqueeze(3).broadcast_to([P, fp, 4, 16]),
                    in1=Q12.rearrange("p q j k -> p q (j k)").unsqueeze(2).broadcast_to([P, fp, 4, 16]),
                    op=A.mult)
                g2v = G.rearrange("p q (m k) d -> p q m k d", m=16)[:, :, :, :, 2]
                nc.vector.tensor_tensor(
                    out=g2v,
                    in0=dbx(2).unsqueeze(2).broadcast_to([P, fp, 16, 4]),
                    in1=p01f.unsqueeze(3).broadcast_to([P, fp, 16, 4]),
                    op=A.mult)

                nc.sync.dma_start(out=sh_v[it], in_=S.rearrange("p q m k -> p (q m k)"))
                nc.sync.dma_start(out=gr_v[it], in_=G.rearrange("p q w d -> p (q w d)"))

    nc.compile()
    return nc


_MODULE_CACHE = {}


def _get_module():
    if "nc" not in _MODULE_CACHE:
        _MODULE_CACHE["nc"] = build_module()
    return _MODULE_CACHE["nc"]


def run(position_stack: np.ndarray, trace: bool = False):
    """Run on 8 NeuronCores. Returns ((shapef, grad), BassKernelResults)."""
    pos = np.ascontiguousarray(np.asarray(position_stack, dtype=np.float32))
    assert pos.shape == (N_TOTAL, 3), pos.shape

    nc = _get_module()
    in_maps = []
    for c in range(N_CORES):
        sl = pos[c * N_PER_CORE : (c + 1) * N_PER_CORE]
        padded = np.full((NPC, 3), 0.5, dtype=np.float32)
        padded[:N_PER_CORE] = sl
        in_maps.append({"pos": padded})

    res = bass_utils.run_bass_kernel_spmd(
        nc, in_maps, core_ids=list(range(N_CORES)), trace=trace,
    )

    shapef = np.empty((N_TOTAL, 64), dtype=np.float32)
    grad = np.empty((N_TOTAL, 64, 3), dtype=np.float32)
    for c, r in enumerate(res.results):
        shapef[c * N_PER_CORE : (c + 1) * N_PER_CORE] = r["shapef"][:N_PER_CORE]
        grad[c * N_PER_CORE : (c + 1) * N_PER_CORE] = r["grad"][:N_PER_CORE].reshape(
            N_PER_CORE, 64, 3
        )
    return (shapef, grad), res


def kernel(position_stack: np.ndarray):
    (shapef, grad), _ = run(position_stack, trace=False)
    return shapef, grad


# revision 12
# speedup vs baseline: 1.2335x; 1.0285x over previous
"""Trainium2 Bass kernel for CubicShapeFunction (500k particles, 4^3 cubic
B-spline stencil), data-parallel over 8 NeuronCores.

Math: rel = pos*20, f = frac(rel) in [0,1), g = 1-f. The stencil distance
for offset o in {0,1,2,3} is f+1-o, which falls in spline branch
c4,c3,c2,c1 respectively, so (h = 20):

  B0 = g^3/6          DB0 = -h g^2/2
  B1 = 2/3 - f^2(1-f/2)   DB1 = f * (1.5h f - 2h)
  B2 = 2/3 - g^2(1-g/2)   DB2 = g * (2h - 1.5h g)
  B3 = f^3/6          DB3 = h f^2/2

outputs:
  shapef[n, k]  = B0[oi]*B1[oj]*B2[ok],        k = oi*16+oj*4+ok  (o-index per dim)
  grad[n, k, d] = DBd[od] * prod_{d'!=d} Bd'[od']

Engine split per tile (128 partitions x fp particles each):
  Scalar(ACT): rel, g, squares, all affine basis-column evaluations
  Vector(DVE): floor trick, cubes, two pair-product-free products,
               S(half), G0, G2
  GpSimd:      pair products P01/P12/Q12, G1, S(other half)
All output tiles are laid out so the S and G DMAs are fully contiguous.
"""

import numpy as np

import concourse.bass as bass  # noqa: F401
import concourse.bacc as bacc
import concourse.tile as tile
from concourse import mybir
from concourse import bass_utils

F32 = mybir.dt.float32
H = 20.0
MAGIC = float(2 ** 23)

N_TOTAL = 500_000
N_CORES = 8
N_PER_CORE = N_TOTAL // N_CORES          # 62500
FP = 56                                   # particles per partition per tile
ROWS_PER_TILE = 128 * FP                  # 7168
NTILES = 9
NPC = ROWS_PER_TILE * NTILES              # 64512 padded rows per core


def build_module(n_rows: int = NPC, fp: int = FP, num_devices: int = N_CORES,
                 enable_asserts: bool = False):
    P = 128
    rows_per_tile = P * fp
    assert n_rows % rows_per_tile == 0
    ntiles = n_rows // rows_per_tile

    nc = bacc.Bacc(
        "TRN2",
        target_bir_lowering=False,
        debug=False,
        enable_asserts=enable_asserts,
        num_devices=num_devices,
    )
    # pre-register activation-bias constants (only 0.0/1.0 exist by default)
    for value in (2.0 / 3.0, -2.0 * H, 2.0 * H):
        t = nc.alloc_sbuf_tensor(f"const-f32-{value}", [128, 1], F32)
        nc.gpsimd.memset(t.ap(), value)
        nc.const_aps.aps[(F32, value)] = t.ap()
    nc.all_engine_barrier()

    pos_d = nc.dram_tensor("pos", [n_rows, 3], F32, kind="ExternalInput").ap()
    shapef_d = nc.dram_tensor("shapef", [n_rows, 64], F32, kind="ExternalOutput").ap()
    grad_d = nc.dram_tensor("grad", [n_rows, 192], F32, kind="ExternalOutput").ap()

    pos_v = pos_d.rearrange("(t p q) d -> p t (q d)", t=ntiles, p=P)
    sh_v = shapef_d.rearrange("(t p q) w -> t p (q w)", t=ntiles, p=P)
    gr_v = grad_d.rearrange("(t p q) w -> t p (q w)", t=ntiles, p=P)

    A = mybir.AluOpType
    ID = mybir.ActivationFunctionType.Identity
    SQ = mybir.ActivationFunctionType.Square

    with tile.TileContext(nc) as tc:
        with (
            tc.tile_pool(name="const", bufs=1) as constp,
            tc.tile_pool(name="inp", bufs=2) as inp,
            tc.tile_pool(name="bdb", bufs=2) as bdbp,
            tc.tile_pool(name="pp", bufs=2) as ppp,
            tc.tile_pool(name="sg", bufs=2) as sgp,
        ):
            # all positions for this core in one DMA (6.7KB/partition)
            POS = constp.tile([P, ntiles, fp * 3], F32)
            nc.sync.dma_start(out=POS, in_=pos_v)

            for it in range(ntiles):
                posv = POS[:, it, :].rearrange("p (q d) -> p q d", d=3)

                def t3(tag):
                    return inp.tile([P, fp, 3], F32, tag=tag, name=tag)

                rel, fl, gt, fr, g = t3("rel"), t3("fl"), t3("gt"), t3("fr"), t3("g")
                f2, g2, f3, g3 = t3("f2"), t3("g2"), t3("f3"), t3("g3")
                t1, t2, vv, ww = t3("t1"), t3("t2"), t3("vv"), t3("ww")

                # rel = 20*pos; floor via fp32 magic round (+2^23-2^23, fix up
                # round>rel), f = rel - floor, g = 1 - f. Exact in f32.
                # frac = rel - round + (round > rel)
                nc.scalar.mul(out=rel, in_=posv, mul=H)
                nc.vector.tensor_scalar(out=fl, in0=rel, scalar1=MAGIC, scalar2=MAGIC,
                                        op0=A.add, op1=A.subtract)
                nc.vector.tensor_tensor(out=gt, in0=fl, in1=rel, op=A.is_gt)
                nc.vector.tensor_tensor(out=fr, in0=rel, in1=fl, op=A.subtract)
                nc.gpsimd.tensor_tensor(out=fr, in0=fr, in1=gt, op=A.add)
                nc.scalar.activation(out=g, in_=fr, func=ID, bias=1.0, scale=-1.0)

                # powers
                nc.scalar.activation(out=f2, in_=fr, func=SQ)
                nc.scalar.activation(out=g2, in_=g, func=SQ)
                nc.gpsimd.tensor_tensor(out=f3, in0=f2, in1=fr, op=A.mult)
                nc.gpsimd.tensor_tensor(out=g3, in0=g2, in1=g, op=A.mult)

                B = bdbp.tile([P, fp, 3, 4], F32, tag="B")
                DB = bdbp.tile([P, fp, 3, 4], F32, tag="DB")

                # basis columns
                nc.scalar.mul(out=B[:, :, :, 0], in_=g3, mul=1.0 / 6.0)
                nc.scalar.mul(out=B[:, :, :, 3], in_=f3, mul=1.0 / 6.0)
                nc.scalar.activation(out=t1, in_=fr, func=ID, bias=1.0, scale=-0.5)
                nc.gpsimd.tensor_tensor(out=t1, in0=f2, in1=t1, op=A.mult)
                nc.scalar.activation(out=B[:, :, :, 1], in_=t1, func=ID,
                                     bias=2.0 / 3.0, scale=-1.0)
                nc.scalar.activation(out=t2, in_=g, func=ID, bias=1.0, scale=-0.5)
                nc.gpsimd.tensor_tensor(out=t2, in0=g2, in1=t2, op=A.mult)
                nc.scalar.activation(out=B[:, :, :, 2], in_=t2, func=ID,
                                     bias=2.0 / 3.0, scale=-1.0)

                # dbasis columns
                nc.scalar.mul(out=DB[:, :, :, 0], in_=g2, mul=-0.5 * H)
                nc.scalar.mul(out=DB[:, :, :, 3], in_=f2, mul=0.5 * H)
                nc.scalar.activation(out=vv, in_=fr, func=ID, bias=-2.0 * H, scale=1.5 * H)
                nc.gpsimd.tensor_tensor(out=DB[:, :, :, 1], in0=vv, in1=fr, op=A.mult)
                nc.scalar.activation(out=ww, in_=g, func=ID, bias=2.0 * H, scale=-1.5 * H)
                nc.gpsimd.tensor_tensor(out=DB[:, :, :, 2], in0=ww, in1=g, op=A.mult)

                def bx(d):
                    return B[:, :, d, :]

                def dbx(d):
                    return DB[:, :, d, :]

                # pair products on gpsimd
                P01 = ppp.tile([P, fp, 4, 4], F32, tag="P01")
                P12 = ppp.tile([P, fp, 4, 4], F32, tag="P12")
                Q12 = ppp.tile([P, fp, 4, 4], F32, tag="Q12")
                nc.gpsimd.tensor_tensor(
                    out=P01,
                    in0=bx(0).unsqueeze(3).broadcast_to([P, fp, 4, 4]),
                    in1=bx(1).unsqueeze(2).broadcast_to([P, fp, 4, 4]),
                    op=A.mult)
                nc.gpsimd.tensor_tensor(
                    out=P12,
                    in0=bx(1).unsqueeze(3).broadcast_to([P, fp, 4, 4]),
                    in1=bx(2).unsqueeze(2).broadcast_to([P, fp, 4, 4]),
                    op=A.mult)
                nc.gpsimd.tensor_tensor(
                    out=Q12,
                    in0=dbx(1).unsqueeze(3).broadcast_to([P, fp, 4, 4]),
                    in1=bx(2).unsqueeze(2).broadcast_to([P, fp, 4, 4]),
                    op=A.mult)

                # final products
                S = sgp.tile([P, fp, 16, 4], F32, tag="S")
                p01f = P01.rearrange("p q i j -> p q (i j)")
                nc.vector.tensor_tensor(
                    out=S,
                    in0=p01f.unsqueeze(3).broadcast_to([P, fp, 16, 4]),
                    in1=bx(2).unsqueeze(2).broadcast_to([P, fp, 16, 4]),
                    op=A.mult)

                G = sgp.tile([P, fp, 64, 3], F32, tag="G")
                g0 = G.rearrange("p q (i m) d -> p q i m d", i=4)[:, :, :, :, 0]
                nc.vector.tensor_tensor(
                    out=g0,
                    in0=dbx(0).unsqueeze(3).broadcast_to([P, fp, 4, 16]),
                    in1=P12.rearrange("p q j k -> p q (j k)").unsqueeze(2).broadcast_to([P, fp, 4, 16]),
                    op=A.mult)
                g1 = G.rearrange("p q (i m) d -> p q i m d", i=4)[:, :, :, :, 1]
                nc.vector.tensor_tensor(
                    out=g1,
                    in0=bx(0).unsqueeze(3).broadcast_to([P, fp, 4, 16]),
                    in1=Q12.rearrange("p q j k -> p q (j k)").unsqueeze(2).broadcast_to([P, fp, 4, 16]),
                    op=A.mult)
                g2v = G.rearrange("p q (m k) d -> p q m k d", m=16)[:, :, :, :, 2]
                nc.vector.tensor_tensor(
                    out=g2v,
                    in0=dbx(2).unsqueeze(2).broadcast_to([P, fp, 16, 4]),
                    in1=p01f.unsqueeze(3).broadcast_to([P, fp, 16, 4]),
                    op=A.mult)

                nc.sync.dma_start(out=sh_v[it], in_=S.rearrange("p q m k -> p (q m k)"))
                nc.sync.dma_start(out=gr_v[it], in_=G.rearrange("p q w d -> p (q w d)"))

    nc.compile()
    return nc


_MODULE_CACHE = {}


def _get_module():
    if "nc" not in _MODULE_CACHE:
        _MODULE_CACHE["nc"] = build_module()
    return _MODULE_CACHE["nc"]


def run(position_stack: np.ndarray, trace: bool = False):
    """Run on 8 NeuronCores. Returns ((shapef, grad), BassKernelResults)."""
    pos = np.ascontiguousarray(np.asarray(position_stack, dtype=np.float32))
    assert pos.shape == (N_TOTAL, 3), pos.shape

    nc = _get_module()
    in_maps = []
    for c in range(N_CORES):
        sl = pos[c * N_PER_CORE : (c + 1) * N_PER_CORE]
        padded = np.full((NPC, 3), 0.5, dtype=np.float32)
        padded[:N_PER_CORE] = sl
        in_maps.append({"pos": padded})

    res = bass_utils.run_bass_kernel_spmd(
        nc, in_maps, core_ids=list(range(N_CORES)), trace=trace,
    )

    shapef = np.empty((N_TOTAL, 64), dtype=np.float32)
    grad = np.empty((N_TOTAL, 64, 3), dtype=np.float32)
    for c, r in enumerate(res.results):
        shapef[c * N_PER_CORE : (c + 1) * N_PER_CORE] = r["shapef"][:N_PER_CORE]
        grad[c * N_PER_CORE : (c + 1) * N_PER_CORE] = r["grad"][:N_PER_CORE].reshape(
            N_PER_CORE, 64, 3
        )
    return (shapef, grad), res


def kernel(position_stack: np.ndarray):
    (shapef, grad), _ = run(position_stack, trace=False)
    return shapef, grad


# revision 16
# speedup vs baseline: 1.3413x; 1.0874x over previous
"""Trainium2 Bass kernel for CubicShapeFunction (500k particles, 4^3 cubic
B-spline stencil), data-parallel over 8 NeuronCores.

Math: rel = pos*20, f = frac(rel) in [0,1), g = 1-f. The stencil distance
for offset o in {0,1,2,3} is f+1-o, which falls in spline branch
c4,c3,c2,c1 respectively, so (h = 20):

  B0 = g^3/6          DB0 = -h g^2/2
  B1 = 2/3 - f^2(1-f/2)   DB1 = f * (1.5h f - 2h)
  B2 = 2/3 - g^2(1-g/2)   DB2 = g * (2h - 1.5h g)
  B3 = f^3/6          DB3 = h f^2/2

outputs:
  shapef[n, k]  = B0[oi]*B1[oj]*B2[ok],        k = oi*16+oj*4+ok  (o-index per dim)
  grad[n, k, d] = DBd[od] * prod_{d'!=d} Bd'[od']

Engine split per tile (128 partitions x fp particles each):
  Scalar(ACT): rel, g, squares, all affine basis-column evaluations
  Vector(DVE): floor trick, cubes, two pair-product-free products,
               S(half), G0, G2
  GpSimd:      pair products P01/P12/Q12, G1, S(other half)
All output tiles are laid out so the S and G DMAs are fully contiguous.
"""

import numpy as np

import concourse.bass as bass  # noqa: F401
import concourse.bacc as bacc
import concourse.tile as tile
from concourse import mybir
from concourse import bass_utils

F32 = mybir.dt.float32
H = 20.0
MAGIC = float(2 ** 23)

N_TOTAL = 500_000
N_CORES = 8
N_PER_CORE = N_TOTAL // N_CORES          # 62500
FP = 56                                   # particles per partition per tile
ROWS_PER_TILE = 128 * FP                  # 7168
NTILES = 9
NPC = ROWS_PER_TILE * NTILES              # 64512 padded rows per core


def build_module(n_rows: int = NPC, fp: int = FP, num_devices: int = N_CORES,
                 enable_asserts: bool = False):
    P = 128
    rows_per_tile = P * fp
    assert n_rows % rows_per_tile == 0
    ntiles = n_rows // rows_per_tile

    nc = bacc.Bacc(
        "TRN2",
        target_bir_lowering=False,
        debug=False,
        enable_asserts=enable_asserts,
        num_devices=num_devices,
    )
    # pre-register activation-bias constants (only 0.0/1.0 exist by default)
    for value in (2.0 / 3.0, -2.0 / 3.0, -2.0 * H / 3.0, 2.0 * H / 3.0):
        t = nc.alloc_sbuf_tensor(f"const-f32-{value}", [128, 1], F32)
        nc.gpsimd.memset(t.ap(), value)
        nc.const_aps.aps[(F32, value)] = t.ap()
    nc.all_engine_barrier()

    pos_d = nc.dram_tensor("pos", [n_rows, 3], F32, kind="ExternalInput").ap()
    shapef_d = nc.dram_tensor("shapef", [n_rows, 64], F32, kind="ExternalOutput").ap()
    grad_d = nc.dram_tensor("grad", [n_rows, 192], F32, kind="ExternalOutput").ap()

    pos_v = pos_d.rearrange("(t p q) d -> p t (q d)", t=ntiles, p=P)
    sh_v = shapef_d.rearrange("(t p q) w -> t p (q w)", t=ntiles, p=P)
    gr_v = grad_d.rearrange("(t p q) w -> t p (q w)", t=ntiles, p=P)

    A = mybir.AluOpType
    ID = mybir.ActivationFunctionType.Identity
    SQ = mybir.ActivationFunctionType.Square

    with tile.TileContext(nc) as tc:
        with (
            tc.tile_pool(name="const", bufs=1) as constp,
            tc.tile_pool(name="inp", bufs=2) as inp,
            tc.tile_pool(name="bdb", bufs=2) as bdbp,
            tc.tile_pool(name="pp", bufs=2) as ppp,
            tc.tile_pool(name="sg", bufs=2) as sgp,
        ):
            # all positions for this core in one DMA (6.7KB/partition)
            POS = constp.tile([P, ntiles, fp * 3], F32)
            nc.sync.dma_start(out=POS, in_=pos_v)

            for it in range(ntiles):
                posv = POS[:, it, :].rearrange("p (q d) -> p q d", d=3)

                def t3(tag):
                    return inp.tile([P, fp, 3], F32, tag=tag, name=tag)

                rel, fl, gt, fr, g = t3("rel"), t3("fl"), t3("gt"), t3("fr"), t3("g")
                f2, g2 = t3("f2"), t3("g2")
                t1, t2, w1, w2 = t3("t1"), t3("t2"), t3("w1"), t3("w2")
                b0t, b3t = t3("b0t"), t3("b3t")

                # rel = 20*pos; floor via fp32 magic round (+2^23-2^23, fix up
                # round>rel), f = rel - floor, g = 1 - f. Exact in f32.
                # frac = rel - round + (round > rel)
                nc.scalar.mul(out=rel, in_=posv, mul=H)
                nc.vector.tensor_scalar(out=fl, in0=rel, scalar1=MAGIC, scalar2=MAGIC,
                                        op0=A.add, op1=A.subtract)
                nc.vector.tensor_tensor(out=gt, in0=fl, in1=rel, op=A.is_gt)
                nc.vector.tensor_tensor(out=fr, in0=rel, in1=fl, op=A.subtract)
                nc.gpsimd.tensor_tensor(out=fr, in0=fr, in1=gt, op=A.add)
                nc.scalar.activation(out=g, in_=fr, func=ID, bias=1.0, scale=-1.0)

                # powers
                nc.scalar.activation(out=f2, in_=fr, func=SQ)
                nc.scalar.activation(out=g2, in_=g, func=SQ)

                B = bdbp.tile([P, fp, 3, 4], F32, tag="B")
                DB = bdbp.tile([P, fp, 3, 4], F32, tag="DB")

                # basis columns. B0=g^3/6, B3=f^3/6 via fused (g2*1/6)*g into
                # contiguous temps, ACT copies into the strided columns.
                nc.vector.scalar_tensor_tensor(out=b0t, in0=g2, scalar=1.0 / 6.0,
                                               in1=g, op0=A.mult, op1=A.mult)
                nc.vector.scalar_tensor_tensor(out=b3t, in0=f2, scalar=1.0 / 6.0,
                                               in1=fr, op0=A.mult, op1=A.mult)
                nc.scalar.copy(out=B[:, :, :, 0], in_=b0t)
                nc.scalar.copy(out=B[:, :, :, 3], in_=b3t)
                nc.scalar.activation(out=t1, in_=fr, func=ID, bias=1.0, scale=-0.5)
                nc.gpsimd.tensor_tensor(out=t1, in0=f2, in1=t1, op=A.mult)
                nc.scalar.activation(out=B[:, :, :, 1], in_=t1, func=ID,
                                     bias=2.0 / 3.0, scale=-1.0)
                nc.scalar.activation(out=t2, in_=g, func=ID, bias=1.0, scale=-0.5)
                nc.gpsimd.tensor_tensor(out=t2, in0=g2, in1=t2, op=A.mult)
                nc.scalar.activation(out=B[:, :, :, 2], in_=t2, func=ID,
                                     bias=2.0 / 3.0, scale=-1.0)

                # dbasis columns, all pure ACT:
                # DB1 = 1.5h f^2 - 2h f = 1.5h (f-2/3)^2 - 2h/3  (square trick)
                # DB2 = 2h g - 1.5h g^2 = -1.5h (g-2/3)^2 + 2h/3
                nc.scalar.mul(out=DB[:, :, :, 0], in_=g2, mul=-0.5 * H)
                nc.scalar.mul(out=DB[:, :, :, 3], in_=f2, mul=0.5 * H)
                nc.scalar.activation(out=w1, in_=fr, func=SQ, bias=-2.0 / 3.0, scale=1.0)
                nc.scalar.activation(out=DB[:, :, :, 1], in_=w1, func=ID,
                                     bias=-2.0 * H / 3.0, scale=1.5 * H)
                nc.scalar.activation(out=w2, in_=g, func=SQ, bias=-2.0 / 3.0, scale=1.0)
                nc.scalar.activation(out=DB[:, :, :, 2], in_=w2, func=ID,
                                     bias=2.0 * H / 3.0, scale=-1.5 * H)

                def bx(d):
                    return B[:, :, d, :]

                def dbx(d):
                    return DB[:, :, d, :]

                # pair products over (oi,oj) on gpsimd:
                # P01=B0xB1 (for S,G2), R01=DB0xB1 (G0), R11=B0xDB1 (G1)
                P01 = ppp.tile([P, fp, 4, 4], F32, tag="P01")
                R01 = ppp.tile([P, fp, 4, 4], F32, tag="R01")
                R11 = ppp.tile([P, fp, 4, 4], F32, tag="R11")
                nc.gpsimd.tensor_tensor(
                    out=P01,
                    in0=bx(0).unsqueeze(3).broadcast_to([P, fp, 4, 4]),
                    in1=bx(1).unsqueeze(2).broadcast_to([P, fp, 4, 4]),
                    op=A.mult)
                nc.gpsimd.tensor_tensor(
                    out=R01,
                    in0=dbx(0).unsqueeze(3).broadcast_to([P, fp, 4, 4]),
                    in1=bx(1).unsqueeze(2).broadcast_to([P, fp, 4, 4]),
                    op=A.mult)
                nc.gpsimd.tensor_tensor(
                    out=R11,
                    in0=bx(0).unsqueeze(3).broadcast_to([P, fp, 4, 4]),
                    in1=dbx(1).unsqueeze(2).broadcast_to([P, fp, 4, 4]),
                    op=A.mult)

                # final products, all in the same (pair[ij] x z-basis[k]) form
                S = sgp.tile([P, fp, 16, 4], F32, tag="S")
                p01f = P01.rearrange("p q i j -> p q (i j)")
                r01f = R01.rearrange("p q i j -> p q (i j)")
                r11f = R11.rearrange("p q i j -> p q (i j)")
                nc.vector.tensor_tensor(
                    out=S,
                    in0=p01f.unsqueeze(3).broadcast_to([P, fp, 16, 4]),
                    in1=bx(2).unsqueeze(2).broadcast_to([P, fp, 16, 4]),
                    op=A.mult)

                G = sgp.tile([P, fp, 64, 3], F32, tag="G")
                gm = G.rearrange("p q (m k) d -> p q m k d", m=16)
                nc.vector.tensor_tensor(
                    out=gm[:, :, :, :, 0],
                    in0=r01f.unsqueeze(3).broadcast_to([P, fp, 16, 4]),
                    in1=bx(2).unsqueeze(2).broadcast_to([P, fp, 16, 4]),
                    op=A.mult)
                nc.vector.tensor_tensor(
                    out=gm[:, :, :, :, 1],
                    in0=r11f.unsqueeze(3).broadcast_to([P, fp, 16, 4]),
                    in1=bx(2).unsqueeze(2).broadcast_to([P, fp, 16, 4]),
                    op=A.mult)
                nc.vector.tensor_tensor(
                    out=gm[:, :, :, :, 2],
                    in0=p01f.unsqueeze(3).broadcast_to([P, fp, 16, 4]),
                    in1=dbx(2).unsqueeze(2).broadcast_to([P, fp, 16, 4]),
                    op=A.mult)

                nc.sync.dma_start(out=sh_v[it], in_=S.rearrange("p q m k -> p (q m k)"))
                nc.sync.dma_start(out=gr_v[it], in_=G.rearrange("p q w d -> p (q w d)"))

    nc.compile()
    return nc


_MODULE_CACHE = {}


def _get_module():
    if "nc" not in _MODULE_CACHE:
        _MODULE_CACHE["nc"] = build_module()
    return _MODULE_CACHE["nc"]


def run(position_stack: np.ndarray, trace: bool = False):
    """Run on 8 NeuronCores. Returns ((shapef, grad), BassKernelResults)."""
    pos = np.ascontiguousarray(np.asarray(position_stack, dtype=np.float32))
    assert pos.shape == (N_TOTAL, 3), pos.shape

    nc = _get_module()
    in_maps = []
    for c in range(N_CORES):
        sl = pos[c * N_PER_CORE : (c + 1) * N_PER_CORE]
        padded = np.full((NPC, 3), 0.5, dtype=np.float32)
        padded[:N_PER_CORE] = sl
        in_maps.append({"pos": padded})

    res = bass_utils.run_bass_kernel_spmd(
        nc, in_maps, core_ids=list(range(N_CORES)), trace=trace,
    )

    shapef = np.empty((N_TOTAL, 64), dtype=np.float32)
    grad = np.empty((N_TOTAL, 64, 3), dtype=np.float32)
    for c, r in enumerate(res.results):
        shapef[c * N_PER_CORE : (c + 1) * N_PER_CORE] = r["shapef"][:N_PER_CORE]
        grad[c * N_PER_CORE : (c + 1) * N_PER_CORE] = r["grad"][:N_PER_CORE].reshape(
            N_PER_CORE, 64, 3
        )
    return (shapef, grad), res


def kernel(position_stack: np.ndarray):
    (shapef, grad), _ = run(position_stack, trace=False)
    return shapef, grad
